# revision 7
# baseline (speedup 1.0000x reference)
"""Trainium2 Bass kernel for nn_NeuralMemory (Titans-style neural memory).

Sharding: 8 cores <-> 8 (batch, head) pairs. Each core runs the full
per-(b,h) pipeline.

The end-to-end time under the axon/PJRT tunnel is dominated by
host<->device transfers (~20-40 MiB/s, ~84 ms/round-trip) and per-call
dispatch, so the I/O plan minimizes bytes, parameter count, and re-trace
work:
  - ONE packed f16 input per core: seq token-quarter + per-head weight
    pack half + per-head bias tail (~0.88 MiB/core);
  - seq is uploaded once (each core gets a distinct token quarter of its
    batch) and AllGathered on-device within the 4-core batch group;
  - per-head weights are uploaded once (half per batch replica, w1 in
    f16, w2T rebuilt by on-device transposes) and AllGathered within the
    2-core (batch0,batch1) pair;
  - ones/identity constants are generated on device;
  - the 4 head partials are summed on device via ReduceScatter, so each
    core downloads only a [128, N] f16 quarter of its batch's output;
  - execution goes through a process-cached jax.jit of the same
    bass_exec custom call that bass_utils.run_bass_kernel_spmd builds
    under axon (run_bass_kernel_spmd rebuilds and retraces it on every
    invocation, ~0.9 s/call), without donated zero output buffers (the
    kernel fully writes its output, so no zero-init upload is needed).
    Any failure falls back to run_bass_kernel_spmd.

Math restructuring (validated vs the jax reference in fp64 at ~8e-6):
  - rmsnorm gains folded into projection weights (host-side).
  - inner-loss grads derived manually at the shared initial fast weights;
    the 2/DH*lr factor is dropped for g1/g2 (Newton-Schulz is
    scale-invariant) and applied only to the gamma grad.
  - Newton-Schulz-5 runs directly in the sigma domain on t = -g/nrm
    (t <- a t + (b A + c A^2) t, A = t t^T): numerically stable in fp16.
  - momentum/decay scans fused per chunk with retrieval (which uses the
    weights from the end of the previous chunk).

Layouts: feature-major [feature, token] activations. fp16 matmul operands
(fp32 PSUM accumulation) except the h_pre matmul which runs in fp32r.
Big token-major packs and the per-chunk normalized grads are staged via
DRAM to stay inside SBUF.
"""
import sys

sys.path.insert(0, "/opt/trn_rl_repo")

import numpy as np

import concourse.bass as bass
import concourse.bacc as bacc
import concourse.mybir as mybir
import concourse.tile as tile
from concourse.bass import ts

F32 = mybir.dt.float32
F32R = mybir.dt.float32r
F16 = mybir.dt.float16

DIM, HEADS, DH, CHUNK = 512, 4, 128, 64
HID = DH * 4
B, N = 2, 2048
NCH = N // CHUNK          # 32 chunks
NTT = N // 512            # 4 token tiles
NSA, NSB, NSC = 3.4445, -4.775, 2.0315
AX = mybir.AluOpType
AF = mybir.ActivationFunctionType
X_AXIS = mybir.AxisListType.X
NGRP = 8                  # chunks per NS group (16 NS instances)

GROUPS = [[0, 1, 2, 3], [4, 5, 6, 7]]       # batch groups (4 heads each)
PAIRS = [[0, 4], [1, 5], [2, 6], [3, 7]]    # same-head pairs across batches

SEQ_ELEMS = 512 * 512                        # one token quarter, [DIM, 512]
WPK_COLS = 1552                              # half of the per-head weight pack
WPK_ELEMS = 128 * WPK_COLS
TAIL_OFF = SEQ_ELEMS + WPK_ELEMS             # biasB(512) bias_md(2) gamma(128)
PACKH_ELEMS = TAIL_OFF + 642                 # 461442 f16


def build(nc):
    d = {}
    d["packh"] = nc.dram_tensor("packh", [PACKH_ELEMS], F16, kind="ExternalInput")
    d["outp"] = nc.dram_tensor("outp", [128, N], F16, kind="ExternalOutput")

    with tile.TileContext(nc) as tc:
        _body(nc, tc, d)
    return nc


def _body(nc, tc, d):
    def dma(out, in_):
        nc.sync.dma_start(out=out, in_=in_)

    consts_cm = tc.tile_pool(name="consts", bufs=1)
    persist_cm = tc.tile_pool(name="persist", bufs=1)
    dram_cm = tc.tile_pool(name="dstage", bufs=1, space="DRAM")
    with consts_cm as consts, persist_cm as persist, dram_cm as dstage:
        # -------- input unpack + on-device de-duplication gathers --------
        packh = d["packh"].ap()
        seqb = dstage.tile([512, 512], F16)
        dma(seqb, packh[0:SEQ_ELEMS].rearrange("(p t) -> p t", p=512))
        wpkb = dstage.tile([128, WPK_COLS], F16)
        dma(wpkb, packh[SEQ_ELEMS:TAIL_OFF].rearrange("(p t) -> p t", p=128))

        seqg = dstage.tile([4, 512, 512], F16)
        nc.gpsimd.collective_compute(
            "AllGather", AX.bypass, replica_groups=GROUPS,
            ins=[seqb.opt()], outs=[seqg.opt()])
        wfull = dstage.tile([2, 128, WPK_COLS], F16)
        nc.gpsimd.collective_compute(
            "AllGather", AX.bypass, replica_groups=PAIRS,
            ins=[wpkb.opt()], outs=[wfull.opt()])

        # ---------------- constants ----------------
        wk_h = consts.tile([128, 4, 128], F16)
        wv_h = consts.tile([128, 4, 128], F16)
        wq_h = consts.tile([128, 4, 128], F16)
        w2_h = consts.tile([128, 4, 128], F16)
        wc_h = consts.tile([128, 512], F16)
        w1h16 = consts.tile([128, 512], F16)
        wsm_h = consts.tile([128, 4, 4], F16)
        dma(wk_h.rearrange("p a b -> p (a b)"), wfull[0][:, 0:512])
        dma(wv_h.rearrange("p a b -> p (a b)"), wfull[0][:, 512:1024])
        dma(wq_h.rearrange("p a b -> p (a b)"), wfull[0][:, 1024:1536])
        dma(w2_h.rearrange("p a b -> p (a b)"), wfull[1][:, 0:512])
        dma(wc_h, wfull[1][:, 512:1024])
        dma(w1h16, wfull[1][:, 1024:1536])
        dma(wsm_h.rearrange("p a b -> p (a b)"), wfull[1][:, 1536:1552])
        w1sb = consts.tile([128, 512], F32)
        nc.vector.tensor_copy(out=w1sb, in_=w1h16)
        w1_r = consts.tile([128, 512], F32R)
        nc.vector.tensor_copy(out=w1_r, in_=w1h16)

        biasB16 = consts.tile([128, 4], F16)
        dma(biasB16, packh[TAIL_OFF:TAIL_OFF + 512].rearrange("(p t) -> p t", p=128))
        biasmd16 = consts.tile([2, 1], F16)
        dma(biasmd16,
            packh[TAIL_OFF + 512:TAIL_OFF + 514].rearrange("(p t) -> p t", p=2))
        gamma16 = consts.tile([128, 1], F16)
        dma(gamma16,
            packh[TAIL_OFF + 514:TAIL_OFF + 642].rearrange("(p t) -> p t", p=128))
        biasB = consts.tile([128, 4], F32)
        nc.vector.tensor_copy(out=biasB, in_=biasB16)
        bias_md = consts.tile([2, 1], F32)
        nc.vector.tensor_copy(out=bias_md, in_=biasmd16)
        gamma = consts.tile([128, 1], F32)
        nc.vector.tensor_copy(out=gamma, in_=gamma16)
        epsT = consts.tile([128, 1], F32)
        nc.vector.memset(epsT, 1e-6)

        ones_col_h = consts.tile([128, 1], F16)
        nc.vector.memset(ones_col_h, 1.0)
        ones_row_h = consts.tile([1, 128], F16)
        nc.vector.memset(ones_row_h, 1.0)
        ident_h = consts.tile([128, 128], F16)
        ones_sq = consts.tile([128, 128], F16)
        nc.vector.memset(ones_sq, 1.0)
        nc.gpsimd.affine_select(out=ident_h, in_=ones_sq, pattern=[[-1, 128]],
                                compare_op=AX.is_equal, fill=0.0,
                                base=0, channel_multiplier=1)

        # w2T rebuilt on device (saves shipping it in the pack)
        w2T_h = consts.tile([128, 512], F16)
        with tc.tile_pool(name="psI", bufs=1, space="PSUM") as psI:
            for j in range(4):
                tw_ps = psI.tile([128, 128], F16, tag="tw", bufs=2)
                nc.tensor.transpose(tw_ps, w2_h[:, j, :], ident_h)
                nc.vector.tensor_copy(out=w2T_h[:, ts(j, 128)], in_=tw_ps)

        # -------- persistent tiles + DRAM staging --------
        qT_h = persist.tile([128, N], F16)
        gateB = persist.tile([128, N], F32)
        mdraw = persist.tile([2, NCH], F32)
        momB = persist.tile([128, NCH], F32)
        decm1B = persist.tile([128, NCH], F32)
        gG = persist.tile([128, NCH], F32)
        kc_st = dstage.tile([64, NCH, 128], F16)
        dhh_st = dstage.tile([64, NCH, 128], F16)
        dhpre_st = dstage.tile([64, NCH, 512], F16)
        hact_st = dstage.tile([64, NCH, 512], F16)
        s1_st = dstage.tile([NCH, 128, 512], F16)
        s2_st = dstage.tile([NCH, 128, 512], F16)
        ccin = dstage.tile([512, N], F16)
        ccout = dstage.tile([128, N], F16)

        # ================= PHASE A: store-side, streamed per token-tile ========
        with tc.tile_pool(name="phA", bufs=1) as pA, \
             tc.tile_pool(name="psA", bufs=1, space="PSUM") as psA:
            for tt in range(NTT):
                tsl = ts(tt, 512)
                seq_t = pA.tile([128, 4, 512], F16, tag="seq_t", bufs=2)
                dma(seq_t, seqg[tt].rearrange("(a p) t -> p a t", p=128))
                # rmsnorm scale
                ss_ps = psA.tile([1, 512], F32, tag="mix", bufs=2)
                for j in range(4):
                    sqs = pA.tile([128, 512], F16, tag="sqs", bufs=2)
                    nc.scalar.activation(out=sqs, in_=seq_t[:, j, :], func=AF.Square)
                    nc.tensor.matmul(ss_ps, ones_col_h, sqs,
                                     start=(j == 0), stop=(j == 3))
                rowt = pA.tile([1, 512], F32, tag="rows", bufs=10)
                nc.scalar.activation(out=rowt, in_=ss_ps, func=AF.Sqrt,
                                     scale=1.0 / DIM, bias=epsT[0:1, :])
                rs_f = pA.tile([1, 512], F32, tag="rows", bufs=10)
                nc.vector.reciprocal(out=rs_f, in_=rowt)
                rs_h = pA.tile([1, 512], F16, tag="rows", bufs=10)
                nc.scalar.copy(out=rs_h, in_=rs_f)
                rsb_ps = psA.tile([128, 512], F32, tag="bc", bufs=2)
                nc.tensor.matmul(rsb_ps, ones_row_h, rs_h, start=True, stop=True)
                sT_t = pA.tile([128, 4, 512], F16, tag="sT_t", bufs=2)
                for j in range(4):
                    nc.vector.tensor_mul(out=sT_t[:, j, :], in0=seq_t[:, j, :],
                                         in1=rsb_ps)

                # projections
                k_ps = psA.tile([128, 512], F32, tag="proj", bufs=2)
                for j in range(4):
                    nc.tensor.matmul(k_ps, wk_h[:, j, :], sT_t[:, j, :],
                                     start=(j == 0), stop=(j == 3))
                kT_r = pA.tile([128, 512], F32R, tag="kT_r")
                nc.vector.tensor_copy(out=kT_r, in_=k_ps)
                kT_h = pA.tile([128, 512], F16, tag="kT_h")
                nc.scalar.copy(out=kT_h, in_=k_ps)
                v_ps = psA.tile([128, 512], F32, tag="proj", bufs=2)
                for j in range(4):
                    nc.tensor.matmul(v_ps, wv_h[:, j, :], sT_t[:, j, :],
                                     start=(j == 0), stop=(j == 3))
                kvT = pA.tile([128, 512], F32, tag="kvT")
                nc.vector.tensor_sub(out=kvT, in0=kT_r.bitcast(F32), in1=v_ps)
                q_ps = psA.tile([128, 512], F32, tag="proj", bufs=2)
                for j in range(4):
                    nc.tensor.matmul(q_ps, wq_h[:, j, :], sT_t[:, j, :],
                                     start=(j == 0), stop=(j == 3))
                nc.scalar.copy(out=qT_h[:, tsl], in_=q_ps)
                sm_ps = psA.tile([4, 512], F32, tag="mix", bufs=2)
                for j in range(4):
                    nc.tensor.matmul(sm_ps, wsm_h[:, j, :], sT_t[:, j, :],
                                     start=(j == 0), stop=(j == 3))
                # copy to sbuf, then extract rows at partition 0 via tiny DMAs
                smsb = pA.tile([4, 512], F32, tag="smsb", bufs=2)
                nc.vector.tensor_copy(out=smsb, in_=sm_ps)
                lr_row = pA.tile([1, 512], F32, tag="rows", bufs=10)
                gt_row = pA.tile([1, 512], F32, tag="rows", bufs=10)
                md_rows = pA.tile([2, 512], F32, tag="md_rows", bufs=2)
                dma(lr_row, smsb[0:1, :])
                dma(gt_row, smsb[3:4, :])
                dma(md_rows, smsb[1:3, :])
                nc.vector.tensor_reduce(
                    out=mdraw[:, tt * 8:(tt + 1) * 8],
                    in_=md_rows.rearrange("p (c k) -> p c k", k=CHUNK),
                    axis=X_AXIS, op=AX.add)
                lr_h = pA.tile([1, 512], F16, tag="rows", bufs=10)
                nc.scalar.copy(out=lr_h, in_=lr_row)
                gt_h = pA.tile([1, 512], F16, tag="rows", bufs=10)
                nc.scalar.copy(out=gt_h, in_=gt_row)
                lg_ps = psA.tile([128, 512], F32, tag="bc", bufs=2)
                nc.tensor.matmul(lg_ps, ones_row_h, lr_h, start=True, stop=True)
                lrB = pA.tile([128, 512], F32, tag="lrB")
                nc.scalar.activation(out=lrB, in_=lg_ps, func=AF.Sigmoid,
                                     bias=biasB[:, 0:1])
                gt_ps = psA.tile([128, 512], F32, tag="bc", bufs=2)
                nc.tensor.matmul(gt_ps, ones_row_h, gt_h, start=True, stop=True)
                nc.scalar.activation(out=gateB[:, tsl], in_=gt_ps, func=AF.Sigmoid)

                # forward MLP (h_pre in fp32r, rest fp16)
                hact_h = pA.tile([128, 4, 512], F16, tag="hact_h")
                dgel = pA.tile([128, 4, 512], F32, tag="dgel")
                for j in range(4):
                    hp_ps = psA.tile([128, 512], F32, tag="proj", bufs=2)
                    nc.tensor.matmul(hp_ps, w1_r[:, ts(j, 128)], kT_r,
                                     start=True, stop=True)
                    nc.scalar.activation(out=hact_h[:, j, :], in_=hp_ps,
                                         func=AF.Gelu)
                    nc.scalar.activation(out=dgel[:, j, :], in_=hp_ps,
                                         func=AF.Derivative_Gelu)
                hh_ps = psA.tile([128, 512], F32, tag="proj", bufs=2)
                for j in range(4):
                    nc.tensor.matmul(hh_ps, w2_h[:, j, :], hact_h[:, j, :],
                                     start=(j == 0), stop=(j == 3))
                hhsb = pA.tile([128, 512], F32, tag="hhsb")
                nc.vector.tensor_copy(out=hhsb, in_=hh_ps)
                sq2 = pA.tile([128, 512], F16, tag="sq2", bufs=2)
                nc.scalar.activation(out=sq2, in_=hh_ps, func=AF.Square)
                ms_ps = psA.tile([1, 512], F32, tag="mix", bufs=2)
                nc.tensor.matmul(ms_ps, ones_col_h, sq2, start=True, stop=True)
                rowt2 = pA.tile([1, 512], F32, tag="rows", bufs=10)
                nc.scalar.activation(out=rowt2, in_=ms_ps, func=AF.Sqrt,
                                     scale=1.0 / DH, bias=epsT[0:1, :])
                srs_f = pA.tile([1, 512], F32, tag="rows", bufs=10)
                nc.vector.reciprocal(out=srs_f, in_=rowt2)
                srs_h = pA.tile([1, 512], F16, tag="rows", bufs=10)
                nc.scalar.copy(out=srs_h, in_=srs_f)
                srsb_ps = psA.tile([128, 512], F32, tag="bc", bufs=2)
                nc.tensor.matmul(srsb_ps, ones_row_h, srs_h, start=True, stop=True)
                ysb = pA.tile([128, 512], F32, tag="ysb")
                nc.vector.tensor_mul(out=ysb, in0=hhsb, in1=srsb_ps)
                dp = pA.tile([128, 512], F32, tag="dp")
                nc.vector.scalar_tensor_tensor(out=dp, in0=ysb, scalar=gamma,
                                               in1=kvT, op0=AX.mult, op1=AX.add)
                nc.vector.tensor_mul(out=dp, in0=dp, in1=lrB)
                gp = pA.tile([128, 512], F32, tag="gp", bufs=2)
                nc.vector.tensor_mul(out=gp, in0=dp, in1=ysb)
                nc.vector.tensor_reduce(out=gG[:, tt * 8:(tt + 1) * 8],
                                        in_=gp.rearrange("p (c k) -> p c k", k=CHUNK),
                                        axis=X_AXIS, op=AX.add)
                dY = pA.tile([128, 512], F32, tag="dY")
                nc.vector.tensor_scalar_mul(out=dY, in0=dp, scalar1=gamma)
                dprod = pA.tile([128, 512], F16, tag="dprod", bufs=2)
                nc.vector.tensor_mul(out=dprod, in0=dY, in1=hhsb)
                dot_ps = psA.tile([1, 512], F32, tag="mix", bufs=2)
                nc.tensor.matmul(dot_ps, ones_col_h, dprod, start=True, stop=True)
                s3 = pA.tile([1, 512], F32, tag="rows", bufs=10)
                nc.vector.tensor_mul(out=s3, in0=srs_f, in1=srs_f)
                nc.vector.tensor_mul(out=s3, in0=s3, in1=srs_f)
                c_f = pA.tile([1, 512], F32, tag="rows", bufs=10)
                nc.vector.tensor_mul(out=c_f, in0=s3, in1=dot_ps)
                c_h = pA.tile([1, 512], F16, tag="rows", bufs=10)
                nc.scalar.activation(out=c_h, in_=c_f, func=AF.Copy, scale=1.0 / DH)
                cb_ps = psA.tile([128, 512], F32, tag="bc", bufs=2)
                nc.tensor.matmul(cb_ps, ones_row_h, c_h, start=True, stop=True)
                m1t = pA.tile([128, 512], F32, tag="m1t", bufs=2)
                nc.vector.tensor_mul(out=m1t, in0=dY, in1=srsb_ps)
                m2t = pA.tile([128, 512], F32, tag="m2t", bufs=2)
                nc.vector.tensor_mul(out=m2t, in0=hhsb, in1=cb_ps)
                dhh_h = pA.tile([128, 512], F16, tag="dhh_h")
                nc.vector.tensor_sub(out=dhh_h, in0=m1t, in1=m2t)

                # backward to dhpre (fp16)
                dhpre_h = pA.tile([128, 4, 512], F16, tag="dhpre_h")
                for j in range(4):
                    da_ps = psA.tile([128, 512], F32, tag="proj", bufs=2)
                    nc.tensor.matmul(da_ps, w2T_h[:, ts(j, 128)], dhh_h,
                                     start=True, stop=True)
                    nc.vector.tensor_mul(out=dhpre_h[:, j, :], in0=da_ps,
                                         in1=dgel[:, j, :])

                # token-major transposes (fp16) -> staging -> chunk-major DRAM
                st_kc = pA.tile([128, 4, 128], F16, tag="st_kc", bufs=1)
                st_dh = pA.tile([128, 4, 128], F16, tag="st_dh", bufs=1)
                st_dp = pA.tile([128, 4, 512], F16, tag="st_dp", bufs=1)
                st_ha = pA.tile([128, 4, 512], F16, tag="st_ha", bufs=1)
                for blk in range(4):
                    bsl = ts(blk, 128)
                    tp_ps = psA.tile([128, 4, 128], F16, tag="tp", bufs=2)
                    nc.tensor.transpose(tp_ps[:, 0, :], kT_h[:, bsl], ident_h)
                    nc.tensor.transpose(tp_ps[:, 1, :], dhh_h[:, bsl], ident_h)
                    nc.vector.tensor_copy(out=st_kc[:, blk, :], in_=tp_ps[:, 0, :])
                    nc.vector.tensor_copy(out=st_dh[:, blk, :], in_=tp_ps[:, 1, :])
                    for j in range(4):
                        t2_ps = psA.tile([128, 4, 128], F16, tag="tp", bufs=2)
                        nc.tensor.transpose(t2_ps[:, 0, :], dhpre_h[:, j, bsl],
                                            ident_h)
                        nc.tensor.transpose(t2_ps[:, 1, :], hact_h[:, j, bsl],
                                            ident_h)
                        nc.vector.tensor_copy(out=st_dp[:, blk, ts(j, 128)],
                                              in_=t2_ps[:, 0, :])
                        nc.vector.tensor_copy(out=st_ha[:, blk, ts(j, 128)],
                                              in_=t2_ps[:, 1, :])
                for cm, stg in [(kc_st, st_kc), (dhh_st, st_dh),
                                (dhpre_st, st_dp), (hact_st, st_ha)]:
                    v = cm.rearrange("p (a two) x -> p a two x", two=2)
                    dma(v[:, 4 * tt:4 * tt + 4, 0, :], stg[0:64, :, :])
                    dma(v[:, 4 * tt:4 * tt + 4, 1, :], stg[64:128, :, :])

            # finish mom/dec (all chunks)
            mds = pA.tile([2, NCH], F32, tag="mds")
            nc.scalar.activation(out=mds, in_=mdraw, func=AF.Sigmoid,
                                 scale=1.0 / CHUNK, bias=bias_md)
            mrow_f = pA.tile([1, NCH], F32, tag="mrow_f")
            drow_f = pA.tile([1, NCH], F32, tag="drow_f")
            dma(mrow_f, mds[0:1, :])
            dma(drow_f, mds[1:2, :])
            mrow = pA.tile([1, NCH], F16, tag="mrow")
            drow = pA.tile([1, NCH], F16, tag="drow")
            nc.scalar.copy(out=mrow, in_=mrow_f)
            nc.scalar.copy(out=drow, in_=drow_f)
            mb_ps = psA.tile([128, 512], F32, tag="bc", bufs=2)
            nc.tensor.matmul(mb_ps[:, 0:NCH], ones_row_h, mrow, start=True, stop=True)
            nc.tensor.matmul(mb_ps[:, 64:64 + NCH], ones_row_h, drow,
                             start=True, stop=True)
            nc.vector.tensor_copy(out=momB, in_=mb_ps[:, 0:NCH])
            nc.scalar.activation(out=decm1B, in_=mb_ps[:, 64:64 + NCH],
                                 func=AF.Identity, scale=-1.0, bias=1.0)
            nc.vector.tensor_scalar_mul(out=gG, in0=gG, scalar1=-2.0 / DH)

        # ================= PHASE B: grads + sigma-domain NS5 =====================
        with tc.tile_pool(name="phB", bufs=1) as pB, \
             tc.tile_pool(name="psB", bufs=1, space="PSUM") as psB:
            for g in range(NCH // NGRP):
                chs = list(range(g * NGRP, (g + 1) * NGRP))
                n_inst = 2 * NGRP
                gsl = ts(g, NGRP)
                kc_g = pB.tile([64, NGRP, 128], F16, tag="kc_g", bufs=2)
                dma(kc_g, kc_st[:, gsl, :])
                dhh_g = pB.tile([64, NGRP, 128], F16, tag="dhh_g", bufs=2)
                dma(dhh_g, dhh_st[:, gsl, :])
                dhpre_g = pB.tile([64, NGRP, 512], F16, tag="dhpre_g", bufs=2)
                dma(dhpre_g, dhpre_st[:, gsl, :])
                hact_g = pB.tile([64, NGRP, 512], F16, tag="hact_g", bufs=2)
                dma(hact_g, hact_st[:, gsl, :])
                R = pB.tile([128, n_inst], F32, tag="R", bufs=2)
                gsb = pB.tile([128, n_inst, 512], F16, tag="gsb", bufs=1)
                for ii, c in enumerate(chs):
                    kc_l = kc_g[:, ii, :]
                    dhp_l = dhpre_g[:, ii, :]
                    dhh_l = dhh_g[:, ii, :]
                    ha_l = hact_g[:, ii, :]
                    g_ps = psB.tile([128, 512], F32, tag="g", bufs=2)
                    nc.tensor.matmul(g_ps, kc_l, dhp_l, start=True, stop=True)
                    nc.vector.tensor_copy(out=gsb[:, 2 * ii, :], in_=g_ps)
                    scr = pB.tile([128, 512], F16, tag="scr", bufs=2)
                    nc.vector.scalar_tensor_tensor(
                        out=scr, in0=gsb[:, 2 * ii, :], scalar=1.0,
                        in1=gsb[:, 2 * ii, :], op0=AX.mult, op1=AX.mult,
                        accum_out=R[:, 2 * ii:2 * ii + 1])
                    g2_ps = psB.tile([128, 512], F32, tag="g", bufs=2)
                    nc.tensor.matmul(g2_ps, dhh_l, ha_l, start=True, stop=True)
                    nc.vector.tensor_copy(out=gsb[:, 2 * ii + 1, :], in_=g2_ps)
                    scr2 = pB.tile([128, 512], F16, tag="scr", bufs=2)
                    nc.vector.scalar_tensor_tensor(
                        out=scr2, in0=gsb[:, 2 * ii + 1, :], scalar=1.0,
                        in1=gsb[:, 2 * ii + 1, :], op0=AX.mult, op1=AX.mult,
                        accum_out=R[:, 2 * ii + 1:2 * ii + 2])
                # norms
                Rh = pB.tile([128, n_inst], F16, tag="Rh", bufs=2)
                nc.vector.tensor_copy(out=Rh, in_=R)
                nrm_ps = psB.tile([1, n_inst], F32, tag="nrm", bufs=2)
                for i2 in range(n_inst):
                    nc.tensor.matmul(nrm_ps[:, i2:i2 + 1], ones_col_h,
                                     Rh[:, i2:i2 + 1], start=True, stop=True)
                inv2 = pB.tile([1, n_inst], F32, tag="inv2", bufs=2)
                nc.vector.reciprocal(out=inv2, in_=nrm_ps)
                ninv = pB.tile([1, n_inst], F32, tag="ninv", bufs=2)
                nc.scalar.activation(out=ninv, in_=inv2, func=AF.Sqrt)
                nc.scalar.activation(out=ninv, in_=ninv, func=AF.Copy, scale=-1.0)
                nb = pB.tile([128, n_inst], F32, tag="nb", bufs=2)
                nc.gpsimd.partition_broadcast(nb, ninv)

                # direct sigma-domain NS5 on t = -g/nrm (fp16, stable)
                for i2 in range(n_inst):
                    c = chs[i2 // 2]
                    tP = pB.tile([128, 512], F16, tag="tP", bufs=2)
                    nc.vector.tensor_scalar_mul(out=tP, in0=gsb[:, i2, :],
                                                scalar1=nb[:, i2:i2 + 1])
                    tT = pB.tile([128, 4, 128], F16, tag="tT", bufs=2)
                    for j in range(4):
                        tt_ps = psB.tile([128, 128], F16, tag="ttp", bufs=2)
                        nc.tensor.transpose(tt_ps, tP[:, ts(j, 128)], ident_h)
                        nc.vector.tensor_copy(out=tT[:, j, :], in_=tt_ps)
                    for k in range(5):
                        A_ps = psB.tile([128, 128], F32, tag="x2", bufs=2)
                        for j in range(4):
                            nc.tensor.matmul(A_ps, tT[:, j, :], tT[:, j, :],
                                             start=(j == 0), stop=(j == 3))
                        Ab = pB.tile([128, 128], F16, tag="Ab", bufs=2)
                        nc.vector.tensor_scalar_mul(out=Ab, in0=A_ps, scalar1=NSB)
                        Au = pB.tile([128, 128], F16, tag="Au", bufs=2)
                        nc.vector.tensor_copy(out=Au, in_=A_ps)
                        A2_ps = psB.tile([128, 128], F32, tag="x2", bufs=2)
                        nc.tensor.matmul(A2_ps, Ab, Au, start=True, stop=True)
                        Bm = pB.tile([128, 128], F16, tag="Bm", bufs=2)
                        # Bm = (b*A2)*(c/b) + b*A = c*A2 + b*A
                        nc.vector.scalar_tensor_tensor(
                            out=Bm, in0=A2_ps, scalar=NSC / NSB, in1=Ab,
                            op0=AX.mult, op1=AX.add)
                        Bt_ps = psB.tile([128, 512], F32, tag="g", bufs=2)
                        nc.tensor.matmul(Bt_ps, Bm, tP, start=True, stop=True)
                        tPn = pB.tile([128, 512], F16, tag="tP", bufs=2)
                        nc.vector.scalar_tensor_tensor(
                            out=tPn, in0=tP, scalar=NSA, in1=Bt_ps,
                            op0=AX.mult, op1=AX.add)
                        tP = tPn
                        if k < 4:
                            tT = pB.tile([128, 4, 128], F16, tag="tT", bufs=2)
                            for j in range(4):
                                tt_ps = psB.tile([128, 128], F16, tag="ttp", bufs=2)
                                nc.tensor.transpose(tt_ps, tP[:, ts(j, 128)],
                                                    ident_h)
                                nc.vector.tensor_copy(out=tT[:, j, :], in_=tt_ps)
                    if i2 % 2 == 0:
                        dma(s1_st[c], tP)
                    else:
                        # matrix 2: store native (hid, dh) layout via transpose
                        s2n = pB.tile([128, 4, 128], F16, tag="s2n", bufs=2)
                        for j in range(4):
                            tt_ps = psB.tile([128, 128], F16, tag="ttp", bufs=2)
                            nc.tensor.transpose(tt_ps, tP[:, ts(j, 128)], ident_h)
                            nc.vector.tensor_copy(out=s2n[:, j, :], in_=tt_ps)
                        dma(s2_st[c], s2n.rearrange("p a b -> p (a b)"))

        # ================= PHASE C: scans + retrieval + output ================
        with tc.tile_pool(name="phC", bufs=1) as pC, \
             tc.tile_pool(name="psC", bufs=1, space="PSUM") as psC:
            u1 = pC.tile([128, 512], F32, tag="u1")
            u2 = pC.tile([128, 4, 128], F32, tag="u2")
            m1s = pC.tile([128, 512], F32, tag="m1s")
            m2s = pC.tile([128, 4, 128], F32, tag="m2s")
            u1h = pC.tile([128, 512], F16, tag="u1h")
            u2h = pC.tile([128, 4, 128], F16, tag="u2h")
            ugv = pC.tile([128, 1], F32, tag="ugv")
            mgv = pC.tile([128, 1], F32, tag="mgv")
            outT = pC.tile([128, N], F16, tag="outT")
            nc.vector.tensor_copy(out=u1, in_=w1sb)
            nc.vector.tensor_copy(out=u2, in_=w2_h)
            nc.vector.tensor_copy(out=u1h, in_=w1h16)
            nc.vector.tensor_copy(out=u2h, in_=w2_h)
            nc.vector.tensor_copy(out=ugv, in_=gamma)
            nc.vector.memset(m1s, 0.0)
            nc.vector.memset(m2s, 0.0)
            nc.vector.memset(mgv, 0.0)

            for c in range(NCH):
                sl = ts(c, CHUNK)
                s1c = pC.tile([128, 512], F16, tag="s1c", bufs=4)
                dma(s1c, s1_st[c])
                s2c = pC.tile([128, 4, 128], F16, tag="s2c", bufs=4)
                dma(s2c.rearrange("p a b -> p (a b)"), s2_st[c])

                # retrieval with pre-update state
                hp_ps = psC.tile([128, 4, CHUNK], F32, tag="hp", bufs=2)
                for j in range(4):
                    nc.tensor.matmul(hp_ps[:, j, :], u1h[:, ts(j, 128)],
                                     qT_h[:, sl], start=True, stop=True)
                ha_c = pC.tile([128, 4, CHUNK], F16, tag="ha_c", bufs=2)
                nc.scalar.activation(out=ha_c, in_=hp_ps, func=AF.Gelu)
                hh_ps = psC.tile([128, CHUNK], F32, tag="csm", bufs=3)
                for j in range(4):
                    nc.tensor.matmul(hh_ps, u2h[:, j, :], ha_c[:, j, :],
                                     start=(j == 0), stop=(j == 3))
                sqc = pC.tile([128, CHUNK], F16, tag="sqc", bufs=2)
                nc.scalar.activation(out=sqc, in_=hh_ps, func=AF.Square)
                ms_ps = psC.tile([1, CHUNK], F32, tag="csm", bufs=3)
                nc.tensor.matmul(ms_ps, ones_col_h, sqc, start=True, stop=True)
                rr = pC.tile([1, CHUNK], F32, tag="rr", bufs=2)
                nc.scalar.activation(out=rr, in_=ms_ps, func=AF.Sqrt,
                                     scale=1.0 / DH, bias=epsT[0:1, :])
                rr2 = pC.tile([1, CHUNK], F32, tag="rr2", bufs=2)
                nc.vector.reciprocal(out=rr2, in_=rr)
                rrh = pC.tile([1, CHUNK], F16, tag="rrh", bufs=2)
                nc.scalar.copy(out=rrh, in_=rr2)
                sb_ps = psC.tile([128, CHUNK], F32, tag="csm", bufs=3)
                nc.tensor.matmul(sb_ps, ones_row_h, rrh, start=True, stop=True)
                hhc = pC.tile([128, CHUNK], F32, tag="hhc", bufs=2)
                nc.scalar.copy(out=hhc, in_=hh_ps)
                yc = pC.tile([128, CHUNK], F32, tag="yc", bufs=2)
                nc.vector.tensor_mul(out=yc, in0=hhc, in1=sb_ps)
                prc = pC.tile([128, CHUNK], F32, tag="prc", bufs=2)
                nc.vector.scalar_tensor_tensor(out=prc, in0=yc, scalar=ugv,
                                               in1=qT_h[:, sl],
                                               op0=AX.mult, op1=AX.add)
                nc.vector.tensor_mul(out=outT[:, sl], in0=prc, in1=gateB[:, sl])

                # scans (s already = NS output)
                nc.vector.scalar_tensor_tensor(out=m1s, in0=m1s,
                                               scalar=momB[:, c:c + 1], in1=s1c,
                                               op0=AX.mult, op1=AX.add)
                nc.vector.scalar_tensor_tensor(out=u1, in0=u1,
                                               scalar=decm1B[:, c:c + 1], in1=m1s,
                                               op0=AX.mult, op1=AX.add)
                nc.scalar.copy(out=u1h, in_=u1)
                nc.vector.scalar_tensor_tensor(out=m2s, in0=m2s,
                                               scalar=momB[:, c:c + 1], in1=s2c,
                                               op0=AX.mult, op1=AX.add)
                nc.vector.scalar_tensor_tensor(out=u2, in0=u2,
                                               scalar=decm1B[:, c:c + 1], in1=m2s,
                                               op0=AX.mult, op1=AX.add)
                nc.scalar.copy(out=u2h, in_=u2)
                nc.vector.scalar_tensor_tensor(out=mgv, in0=mgv,
                                               scalar=momB[:, c:c + 1],
                                               in1=gG[:, c:c + 1],
                                               op0=AX.mult, op1=AX.add)
                nc.vector.scalar_tensor_tensor(out=ugv, in0=ugv,
                                               scalar=decm1B[:, c:c + 1], in1=mgv,
                                               op0=AX.mult, op1=AX.add)

            # final projection -> f16 partial, staged to DRAM for ReduceScatter
            for i in range(4):
                for tt in range(NTT):
                    o_ps = psC.tile([128, 512], F32, tag="sps", bufs=3)
                    nc.tensor.matmul(o_ps, wc_h[:, ts(i, 128)], outT[:, ts(tt, 512)],
                                     start=True, stop=True)
                    osb = pC.tile([128, 512], F16, tag="osb", bufs=3)
                    nc.scalar.copy(out=osb, in_=o_ps)
                    dma(ccin[ts(i, 128), ts(tt, 512)], osb)

            # on-device head sum: each core keeps a [128, N] row-quarter
            nc.gpsimd.collective_compute(
                "ReduceScatter", AX.add, replica_groups=GROUPS,
                ins=[ccin.opt()], outs=[ccout.opt()])
            dma(d["outp"].ap(), ccout)


# ------------------- host side -------------------

def _prep_global(inputs):
    """Build the concatenated 8-core packh global [8*PACKH_ELEMS] f16,
    writing each region in place (single cast/copy pass per datum)."""
    f32, f16 = np.float32, np.float16
    sg = np.asarray(inputs["store_g"], f32)[:, None]
    rg = np.asarray(inputs["retrieve_g"], f32)[:, None]

    def tile128(w):  # (512, X) -> rows grouped as (128, 4, X) -> (128, 4*X)
        w = np.asarray(w, f32)
        return np.ascontiguousarray(
            w.reshape(4, 128, -1).transpose(1, 0, 2).reshape(128, -1))

    seq = np.asarray(inputs["seq"], f32)
    g = np.empty((8, PACKH_ELEMS), f16)
    half0, half1, tails = [], [], []
    for h in range(HEADS):
        hs = slice(h * DH, (h + 1) * DH)
        wk = tile128(sg * np.asarray(inputs["Wk"], f32)[:, hs]).astype(f16)
        wv = tile128(sg * np.asarray(inputs["Wv"], f32)[:, hs]).astype(f16)
        wq = tile128(rg * np.asarray(inputs["Wq"], f32)[:, hs]).astype(f16)
        wsm = tile128(np.stack([
            sg[:, 0] * np.asarray(inputs["W_lr"], f32)[:, h],
            sg[:, 0] * np.asarray(inputs["Wm"], f32)[:, h],
            sg[:, 0] * np.asarray(inputs["Wd"], f32)[:, h],
            rg[:, 0] * np.asarray(inputs["Wgate"], f32)[:, h]], axis=1)).astype(f16)
        w2 = tile128(np.asarray(inputs["mw2"], f32)[h]).astype(f16)
        wc = np.ascontiguousarray(np.asarray(inputs["Wc"], f32)[hs, :]).astype(f16)
        w1 = np.asarray(inputs["mw1"], f32)[h].astype(f16)
        h0 = np.empty((128, WPK_COLS), f16)
        h0[:, 0:512] = wk; h0[:, 512:1024] = wv; h0[:, 1024:1536] = wq
        h0[:, 1536:1552] = 0.0
        h1 = np.empty((128, WPK_COLS), f16)
        h1[:, 0:512] = w2; h1[:, 512:1024] = wc; h1[:, 1024:1536] = w1
        h1[:, 1536:1552] = wsm
        half0.append(h0)
        half1.append(h1)
        tail = np.empty(642, f16)
        tail[0:512] = 0.0
        tail[0:512].reshape(128, 4)[:, 0] = np.float16(
            np.asarray(inputs["b_lr"], f32)[h])
        tail[512] = np.float16(np.asarray(inputs["bm"], f32)[h])
        tail[513] = np.float16(np.asarray(inputs["bd"], f32)[h])
        tail[514:642] = np.asarray(inputs["mgamma"], f32)[h].astype(f16)
        tails.append(tail)

    for c in range(8):
        b, h = c // HEADS, c % HEADS
        v = g[c]
        # cast+transpose straight into the pack region
        v[0:SEQ_ELEMS].reshape(512, 512)[:] = seq[b][512 * h:512 * (h + 1), :].T
        v[SEQ_ELEMS:TAIL_OFF] = (half0[h] if b == 0 else half1[h]).ravel()
        v[TAIL_OFF:] = tails[h]
    return g


def _prep_in_maps(inputs):
    g = _prep_global(inputs)
    return [{"packh": g[c].copy()} for c in range(8)]


_CACHE = {}


def _get_module():
    if "nc" not in _CACHE:
        nc = bacc.Bacc("TRN2", target_bir_lowering=False, debug=False,
                       num_devices=8)
        build(nc)
        nc.compile()
        _CACHE["nc"] = nc
    return _CACHE["nc"]


def _get_executor(g_example):
    """Process-cached sharded executable of the bass_exec custom call.

    Semantics match bass_utils.run_bass_kernel_spmd under axon
    (bass2jax.run_bass_via_pjrt), except: the executable is built once
    (the library rebuilds + retraces its jit per call, ~0.9 s), no zero
    output buffers are donated (the kernel fully writes outp, so
    uninitialized result buffers are fine and the zero upload is
    skipped), and the AOT compile goes through fast_dispatch_compile
    (C++ fast-path dispatch) when available.
    """
    if "exec" in _CACHE:
        return _CACHE["exec"]
    import jax
    import jax.core
    from jax.sharding import Mesh, PartitionSpec
    try:
        from jax.experimental.shard_map import shard_map
    except ImportError:  # newer jax
        from jax import shard_map
    from concourse import bass2jax

    nc = _get_module()
    bass2jax.install_neuronx_cc_hook()
    partition_name = (nc.partition_id_tensor.name
                      if nc.partition_id_tensor else None)
    in_names, out_names, out_avals = [], [], []
    for alloc in nc.m.functions[0].allocations:
        if not isinstance(alloc, mybir.MemoryLocationSet):
            continue
        name = alloc.memorylocations[0].name
        if alloc.kind == "ExternalInput":
            if name != partition_name:
                in_names.append(name)
        elif alloc.kind == "ExternalOutput":
            out_names.append(name)
            out_avals.append(jax.core.ShapedArray(
                tuple(alloc.tensor_shape), mybir.dt.np(alloc.dtype)))
    assert in_names == ["packh"], in_names
    bind_names = in_names + ([partition_name] if partition_name else [])

    def _body(*args):
        ops = list(args)
        if partition_name is not None:
            ops.append(bass2jax.partition_id_tensor())
        return tuple(bass2jax._bass_exec_p.bind(
            *ops, out_avals=tuple(out_avals), in_names=tuple(bind_names),
            out_names=tuple(out_names), lowering_input_output_aliases=(),
            sim_require_finite=True, sim_require_nnan=True, nc=nc))

    devices = jax.devices()[:8]
    assert len(devices) == 8, f"need 8 devices, got {len(jax.devices())}"
    mesh = Mesh(np.asarray(devices), ("core",))
    shmapped = shard_map(_body, mesh=mesh,
                         in_specs=(PartitionSpec("core"),) * len(in_names),
                         out_specs=(PartitionSpec("core"),) * len(out_names),
                         check_rep=False)
    try:
        sharded = bass2jax.fast_dispatch_compile(
            lambda: jax.jit(shmapped, keep_unused=True)
            .lower(g_example).compile())
    except Exception:
        sharded = jax.jit(shmapped, keep_unused=True)
    _CACHE["exec"] = (sharded, in_names, out_names)
    return _CACHE["exec"]


def _run_fast(g):
    gflat = g.reshape(-1)
    sharded, in_names, out_names = _get_executor(gflat)
    out_arrs = sharded(gflat)
    return {nm: np.asarray(out_arrs[i]) for i, nm in enumerate(out_names)}


def kernel(**inputs):
    nc = _get_module()
    g = _prep_global(inputs)
    try:
        outg = _run_fast(g)["outp"]                # [8*128, N] f16
    except Exception:
        from concourse.bass_utils import run_bass_kernel_spmd
        in_maps = [{"packh": g[c].copy()} for c in range(8)]
        res = run_bass_kernel_spmd(nc, in_maps, core_ids=list(range(8)))
        outg = np.concatenate(
            [res.results[c]["outp"] for c in range(8)], axis=0)
    out = np.zeros((B, N, DIM), np.float32)
    for b in range(B):
        out[b] = outg[512 * b:512 * (b + 1)].T.astype(np.float32)
    return out


if __name__ == "__main__":
    dd = np.load("/root/problem/ref_inputs.npz")
    inputs = {k: dd[k] for k in dd.files}
    out = kernel(**inputs)
    exp = np.load("/root/problem/ref_expected.npy")
    err = np.abs(out - exp).max() / np.abs(exp).max()
    rel = np.linalg.norm(out - exp) / np.linalg.norm(exp)
    print(f"absmax-rel: {err:.3e}  l2-rel: {rel:.3e}")


# revision 8
# speedup vs baseline: 1.1862x; 1.1862x over previous
"""Trainium2 Bass kernel for nn_NeuralMemory (Titans-style neural memory).

Sharding: 8 cores <-> 8 (batch, head) pairs. Each core runs the full
per-(b,h) pipeline.

The end-to-end time under the axon/PJRT tunnel is dominated by
host<->device transfers (~20-40 MiB/s, ~84 ms/round-trip) and per-call
dispatch, so the I/O plan minimizes bytes, parameter count, and re-trace
work:
  - ONE packed f16 input per core: seq token-quarter + per-head weight
    pack half + per-head bias tail (~0.88 MiB/core);
  - seq is uploaded once (each core gets a distinct token quarter of its
    batch) and AllGathered on-device within the 4-core batch group;
  - per-head weights are uploaded once (half per batch replica, w1 in
    f16, w2T rebuilt by on-device transposes) and AllGathered within the
    2-core (batch0,batch1) pair;
  - ones/identity constants are generated on device;
  - the 4 head partials are summed on device via ReduceScatter, so each
    core downloads only a [128, N] f16 quarter of its batch's output;
  - execution goes through a process-cached jax.jit of the same
    bass_exec custom call that bass_utils.run_bass_kernel_spmd builds
    under axon (run_bass_kernel_spmd rebuilds and retraces it on every
    invocation, ~0.9 s/call), without donated zero output buffers (the
    kernel fully writes its output, so no zero-init upload is needed).
    Any failure falls back to run_bass_kernel_spmd.

Math restructuring (validated vs the jax reference in fp64 at ~8e-6):
  - rmsnorm gains folded into projection weights (host-side).
  - inner-loss grads derived manually at the shared initial fast weights;
    the 2/DH*lr factor is dropped for g1/g2 (Newton-Schulz is
    scale-invariant) and applied only to the gamma grad.
  - Newton-Schulz-5 runs directly in the sigma domain on t = -g/nrm
    (t <- a t + (b A + c A^2) t, A = t t^T): numerically stable in fp16.
  - momentum/decay scans fused per chunk with retrieval (which uses the
    weights from the end of the previous chunk).

Layouts: feature-major [feature, token] activations. fp16 matmul operands
(fp32 PSUM accumulation) except the h_pre matmul which runs in fp32r.
Big token-major packs and the per-chunk normalized grads are staged via
DRAM to stay inside SBUF.
"""
import sys

sys.path.insert(0, "/opt/trn_rl_repo")

import numpy as np

import concourse.bass as bass
import concourse.bacc as bacc
import concourse.mybir as mybir
import concourse.tile as tile
from concourse.bass import ts

F32 = mybir.dt.float32
F32R = mybir.dt.float32r
F16 = mybir.dt.float16

DIM, HEADS, DH, CHUNK = 512, 4, 128, 64
HID = DH * 4
B, N = 2, 2048
NCH = N // CHUNK          # 32 chunks
NTT = N // 512            # 4 token tiles
NSA, NSB, NSC = 3.4445, -4.775, 2.0315
AX = mybir.AluOpType
AF = mybir.ActivationFunctionType
X_AXIS = mybir.AxisListType.X
NGRP = 8                  # chunks per NS group (16 NS instances)

GROUPS = [[0, 1, 2, 3], [4, 5, 6, 7]]       # batch groups (4 heads each)
PAIRS = [[0, 4], [1, 5], [2, 6], [3, 7]]    # same-head pairs across batches

SEQ_ELEMS = 512 * 512                        # one token quarter, [DIM, 512]
WPK_COLS = 1552                              # half of the per-head weight pack
WPK_ELEMS = 128 * WPK_COLS
TAIL_OFF = SEQ_ELEMS + WPK_ELEMS             # biasB(512) bias_md(2) gamma(128)
PACKH_ELEMS = TAIL_OFF + 642                 # 461442 f16


def build(nc):
    d = {}
    d["packh"] = nc.dram_tensor("packh", [PACKH_ELEMS], F16, kind="ExternalInput")
    d["outp"] = nc.dram_tensor("outp", [128, N], F16, kind="ExternalOutput")

    with tile.TileContext(nc) as tc:
        _body(nc, tc, d)
    return nc


def _body(nc, tc, d):
    def dma(out, in_):
        nc.sync.dma_start(out=out, in_=in_)

    consts_cm = tc.tile_pool(name="consts", bufs=1)
    persist_cm = tc.tile_pool(name="persist", bufs=1)
    dram_cm = tc.tile_pool(name="dstage", bufs=1, space="DRAM")
    with consts_cm as consts, persist_cm as persist, dram_cm as dstage:
        # -------- input unpack + on-device de-duplication gathers --------
        packh = d["packh"].ap()
        seqb = dstage.tile([512, 512], F16)
        dma(seqb, packh[0:SEQ_ELEMS].rearrange("(p t) -> p t", p=512))
        wpkb = dstage.tile([128, WPK_COLS], F16)
        dma(wpkb, packh[SEQ_ELEMS:TAIL_OFF].rearrange("(p t) -> p t", p=128))

        seqg = dstage.tile([4, 512, 512], F16)
        nc.gpsimd.collective_compute(
            "AllGather", AX.bypass, replica_groups=GROUPS,
            ins=[seqb.opt()], outs=[seqg.opt()])
        wfull = dstage.tile([2, 128, WPK_COLS], F16)
        nc.gpsimd.collective_compute(
            "AllGather", AX.bypass, replica_groups=PAIRS,
            ins=[wpkb.opt()], outs=[wfull.opt()])

        # ---------------- constants ----------------
        wk_h = consts.tile([128, 4, 128], F16)
        wv_h = consts.tile([128, 4, 128], F16)
        wq_h = consts.tile([128, 4, 128], F16)
        w2_h = consts.tile([128, 4, 128], F16)
        wc_h = consts.tile([128, 512], F16)
        w1h16 = consts.tile([128, 512], F16)
        wsm_h = consts.tile([128, 4, 4], F16)
        dma(wk_h.rearrange("p a b -> p (a b)"), wfull[0][:, 0:512])
        dma(wv_h.rearrange("p a b -> p (a b)"), wfull[0][:, 512:1024])
        dma(wq_h.rearrange("p a b -> p (a b)"), wfull[0][:, 1024:1536])
        dma(w2_h.rearrange("p a b -> p (a b)"), wfull[1][:, 0:512])
        dma(wc_h, wfull[1][:, 512:1024])
        dma(w1h16, wfull[1][:, 1024:1536])
        dma(wsm_h.rearrange("p a b -> p (a b)"), wfull[1][:, 1536:1552])
        w1sb = consts.tile([128, 512], F32)
        nc.vector.tensor_copy(out=w1sb, in_=w1h16)
        w1_r = consts.tile([128, 512], F32R)
        nc.vector.tensor_copy(out=w1_r, in_=w1h16)

        biasB16 = consts.tile([128, 4], F16)
        dma(biasB16, packh[TAIL_OFF:TAIL_OFF + 512].rearrange("(p t) -> p t", p=128))
        biasmd16 = consts.tile([2, 1], F16)
        dma(biasmd16,
            packh[TAIL_OFF + 512:TAIL_OFF + 514].rearrange("(p t) -> p t", p=2))
        gamma16 = consts.tile([128, 1], F16)
        dma(gamma16,
            packh[TAIL_OFF + 514:TAIL_OFF + 642].rearrange("(p t) -> p t", p=128))
        biasB = consts.tile([128, 4], F32)
        nc.vector.tensor_copy(out=biasB, in_=biasB16)
        bias_md = consts.tile([2, 1], F32)
        nc.vector.tensor_copy(out=bias_md, in_=biasmd16)
        gamma = consts.tile([128, 1], F32)
        nc.vector.tensor_copy(out=gamma, in_=gamma16)
        epsT = consts.tile([128, 1], F32)
        nc.vector.memset(epsT, 1e-6)

        ones_col_h = consts.tile([128, 1], F16)
        nc.vector.memset(ones_col_h, 1.0)
        ones_row_h = consts.tile([1, 128], F16)
        nc.vector.memset(ones_row_h, 1.0)
        ident_h = consts.tile([128, 128], F16)
        ones_sq = consts.tile([128, 128], F16)
        nc.vector.memset(ones_sq, 1.0)
        nc.gpsimd.affine_select(out=ident_h, in_=ones_sq, pattern=[[-1, 128]],
                                compare_op=AX.is_equal, fill=0.0,
                                base=0, channel_multiplier=1)

        # w2T rebuilt on device (saves shipping it in the pack)
        w2T_h = consts.tile([128, 512], F16)
        with tc.tile_pool(name="psI", bufs=1, space="PSUM") as psI:
            for j in range(4):
                tw_ps = psI.tile([128, 128], F16, tag="tw", bufs=2)
                nc.tensor.transpose(tw_ps, w2_h[:, j, :], ident_h)
                nc.vector.tensor_copy(out=w2T_h[:, ts(j, 128)], in_=tw_ps)

        # -------- persistent tiles + DRAM staging --------
        qT_h = persist.tile([128, N], F16)
        gateB = persist.tile([128, N], F32)
        mdraw = persist.tile([2, NCH], F32)
        momB = persist.tile([128, NCH], F32)
        decm1B = persist.tile([128, NCH], F32)
        gG = persist.tile([128, NCH], F32)
        kc_st = dstage.tile([64, NCH, 128], F16)
        dhh_st = dstage.tile([64, NCH, 128], F16)
        dhpre_st = dstage.tile([64, NCH, 512], F16)
        hact_st = dstage.tile([64, NCH, 512], F16)
        s1_st = dstage.tile([NCH, 128, 512], F16)
        s2_st = dstage.tile([NCH, 128, 512], F16)
        ccin = dstage.tile([512, N], F16)
        ccout = dstage.tile([128, N], F16)

        # ================= PHASE A: store-side, streamed per token-tile ========
        with tc.tile_pool(name="phA", bufs=1) as pA, \
             tc.tile_pool(name="psA", bufs=1, space="PSUM") as psA:
            for tt in range(NTT):
                tsl = ts(tt, 512)
                seq_t = pA.tile([128, 4, 512], F16, tag="seq_t", bufs=2)
                dma(seq_t, seqg[tt].rearrange("(a p) t -> p a t", p=128))
                # rmsnorm scale
                ss_ps = psA.tile([1, 512], F32, tag="mix", bufs=2)
                for j in range(4):
                    sqs = pA.tile([128, 512], F16, tag="sqs", bufs=2)
                    nc.scalar.activation(out=sqs, in_=seq_t[:, j, :], func=AF.Square)
                    nc.tensor.matmul(ss_ps, ones_col_h, sqs,
                                     start=(j == 0), stop=(j == 3))
                rowt = pA.tile([1, 512], F32, tag="rows", bufs=10)
                nc.scalar.activation(out=rowt, in_=ss_ps, func=AF.Sqrt,
                                     scale=1.0 / DIM, bias=epsT[0:1, :])
                rs_f = pA.tile([1, 512], F32, tag="rows", bufs=10)
                nc.vector.reciprocal(out=rs_f, in_=rowt)
                rs_h = pA.tile([1, 512], F16, tag="rows", bufs=10)
                nc.scalar.copy(out=rs_h, in_=rs_f)
                rsb_ps = psA.tile([128, 512], F32, tag="bc", bufs=2)
                nc.tensor.matmul(rsb_ps, ones_row_h, rs_h, start=True, stop=True)
                sT_t = pA.tile([128, 4, 512], F16, tag="sT_t", bufs=2)
                for j in range(4):
                    nc.vector.tensor_mul(out=sT_t[:, j, :], in0=seq_t[:, j, :],
                                         in1=rsb_ps)

                # projections
                k_ps = psA.tile([128, 512], F32, tag="proj", bufs=2)
                for j in range(4):
                    nc.tensor.matmul(k_ps, wk_h[:, j, :], sT_t[:, j, :],
                                     start=(j == 0), stop=(j == 3))
                kT_r = pA.tile([128, 512], F32R, tag="kT_r")
                nc.vector.tensor_copy(out=kT_r, in_=k_ps)
                kT_h = pA.tile([128, 512], F16, tag="kT_h")
                nc.scalar.copy(out=kT_h, in_=k_ps)
                v_ps = psA.tile([128, 512], F32, tag="proj", bufs=2)
                for j in range(4):
                    nc.tensor.matmul(v_ps, wv_h[:, j, :], sT_t[:, j, :],
                                     start=(j == 0), stop=(j == 3))
                kvT = pA.tile([128, 512], F32, tag="kvT")
                nc.vector.tensor_sub(out=kvT, in0=kT_r.bitcast(F32), in1=v_ps)
                q_ps = psA.tile([128, 512], F32, tag="proj", bufs=2)
                for j in range(4):
                    nc.tensor.matmul(q_ps, wq_h[:, j, :], sT_t[:, j, :],
                                     start=(j == 0), stop=(j == 3))
                nc.scalar.copy(out=qT_h[:, tsl], in_=q_ps)
                sm_ps = psA.tile([4, 512], F32, tag="mix", bufs=2)
                for j in range(4):
                    nc.tensor.matmul(sm_ps, wsm_h[:, j, :], sT_t[:, j, :],
                                     start=(j == 0), stop=(j == 3))
                # copy to sbuf, then extract rows at partition 0 via tiny DMAs
                smsb = pA.tile([4, 512], F32, tag="smsb", bufs=2)
                nc.vector.tensor_copy(out=smsb, in_=sm_ps)
                lr_row = pA.tile([1, 512], F32, tag="rows", bufs=10)
                gt_row = pA.tile([1, 512], F32, tag="rows", bufs=10)
                md_rows = pA.tile([2, 512], F32, tag="md_rows", bufs=2)
                dma(lr_row, smsb[0:1, :])
                dma(gt_row, smsb[3:4, :])
                dma(md_rows, smsb[1:3, :])
                nc.vector.tensor_reduce(
                    out=mdraw[:, tt * 8:(tt + 1) * 8],
                    in_=md_rows.rearrange("p (c k) -> p c k", k=CHUNK),
                    axis=X_AXIS, op=AX.add)
                lr_h = pA.tile([1, 512], F16, tag="rows", bufs=10)
                nc.scalar.copy(out=lr_h, in_=lr_row)
                gt_h = pA.tile([1, 512], F16, tag="rows", bufs=10)
                nc.scalar.copy(out=gt_h, in_=gt_row)
                lg_ps = psA.tile([128, 512], F32, tag="bc", bufs=2)
                nc.tensor.matmul(lg_ps, ones_row_h, lr_h, start=True, stop=True)
                lrB = pA.tile([128, 512], F32, tag="lrB")
                nc.scalar.activation(out=lrB, in_=lg_ps, func=AF.Sigmoid,
                                     bias=biasB[:, 0:1])
                gt_ps = psA.tile([128, 512], F32, tag="bc", bufs=2)
                nc.tensor.matmul(gt_ps, ones_row_h, gt_h, start=True, stop=True)
                nc.scalar.activation(out=gateB[:, tsl], in_=gt_ps, func=AF.Sigmoid)

                # forward MLP (h_pre in fp32r, rest fp16)
                hact_h = pA.tile([128, 4, 512], F16, tag="hact_h")
                dgel = pA.tile([128, 4, 512], F32, tag="dgel")
                for j in range(4):
                    hp_ps = psA.tile([128, 512], F32, tag="proj", bufs=2)
                    nc.tensor.matmul(hp_ps, w1_r[:, ts(j, 128)], kT_r,
                                     start=True, stop=True)
                    nc.scalar.activation(out=hact_h[:, j, :], in_=hp_ps,
                                         func=AF.Gelu)
                    nc.scalar.activation(out=dgel[:, j, :], in_=hp_ps,
                                         func=AF.Derivative_Gelu)
                hh_ps = psA.tile([128, 512], F32, tag="proj", bufs=2)
                for j in range(4):
                    nc.tensor.matmul(hh_ps, w2_h[:, j, :], hact_h[:, j, :],
                                     start=(j == 0), stop=(j == 3))
                hhsb = pA.tile([128, 512], F32, tag="hhsb")
                nc.vector.tensor_copy(out=hhsb, in_=hh_ps)
                sq2 = pA.tile([128, 512], F16, tag="sq2", bufs=2)
                nc.scalar.activation(out=sq2, in_=hh_ps, func=AF.Square)
                ms_ps = psA.tile([1, 512], F32, tag="mix", bufs=2)
                nc.tensor.matmul(ms_ps, ones_col_h, sq2, start=True, stop=True)
                rowt2 = pA.tile([1, 512], F32, tag="rows", bufs=10)
                nc.scalar.activation(out=rowt2, in_=ms_ps, func=AF.Sqrt,
                                     scale=1.0 / DH, bias=epsT[0:1, :])
                srs_f = pA.tile([1, 512], F32, tag="rows", bufs=10)
                nc.vector.reciprocal(out=srs_f, in_=rowt2)
                srs_h = pA.tile([1, 512], F16, tag="rows", bufs=10)
                nc.scalar.copy(out=srs_h, in_=srs_f)
                srsb_ps = psA.tile([128, 512], F32, tag="bc", bufs=2)
                nc.tensor.matmul(srsb_ps, ones_row_h, srs_h, start=True, stop=True)
                ysb = pA.tile([128, 512], F32, tag="ysb")
                nc.vector.tensor_mul(out=ysb, in0=hhsb, in1=srsb_ps)
                dp = pA.tile([128, 512], F32, tag="dp")
                nc.vector.scalar_tensor_tensor(out=dp, in0=ysb, scalar=gamma,
                                               in1=kvT, op0=AX.mult, op1=AX.add)
                nc.vector.tensor_mul(out=dp, in0=dp, in1=lrB)
                gp = pA.tile([128, 512], F32, tag="gp", bufs=2)
                nc.vector.tensor_mul(out=gp, in0=dp, in1=ysb)
                nc.vector.tensor_reduce(out=gG[:, tt * 8:(tt + 1) * 8],
                                        in_=gp.rearrange("p (c k) -> p c k", k=CHUNK),
                                        axis=X_AXIS, op=AX.add)
                dY = pA.tile([128, 512], F32, tag="dY")
                nc.vector.tensor_scalar_mul(out=dY, in0=dp, scalar1=gamma)
                dprod = pA.tile([128, 512], F16, tag="dprod", bufs=2)
                nc.vector.tensor_mul(out=dprod, in0=dY, in1=hhsb)
                dot_ps = psA.tile([1, 512], F32, tag="mix", bufs=2)
                nc.tensor.matmul(dot_ps, ones_col_h, dprod, start=True, stop=True)
                s3 = pA.tile([1, 512], F32, tag="rows", bufs=10)
                nc.vector.tensor_mul(out=s3, in0=srs_f, in1=srs_f)
                nc.vector.tensor_mul(out=s3, in0=s3, in1=srs_f)
                c_f = pA.tile([1, 512], F32, tag="rows", bufs=10)
                nc.vector.tensor_mul(out=c_f, in0=s3, in1=dot_ps)
                c_h = pA.tile([1, 512], F16, tag="rows", bufs=10)
                nc.scalar.activation(out=c_h, in_=c_f, func=AF.Copy, scale=1.0 / DH)
                cb_ps = psA.tile([128, 512], F32, tag="bc", bufs=2)
                nc.tensor.matmul(cb_ps, ones_row_h, c_h, start=True, stop=True)
                m1t = pA.tile([128, 512], F32, tag="m1t", bufs=2)
                nc.vector.tensor_mul(out=m1t, in0=dY, in1=srsb_ps)
                m2t = pA.tile([128, 512], F32, tag="m2t", bufs=2)
                nc.vector.tensor_mul(out=m2t, in0=hhsb, in1=cb_ps)
                dhh_h = pA.tile([128, 512], F16, tag="dhh_h")
                nc.vector.tensor_sub(out=dhh_h, in0=m1t, in1=m2t)

                # backward to dhpre (fp16)
                dhpre_h = pA.tile([128, 4, 512], F16, tag="dhpre_h")
                for j in range(4):
                    da_ps = psA.tile([128, 512], F32, tag="proj", bufs=2)
                    nc.tensor.matmul(da_ps, w2T_h[:, ts(j, 128)], dhh_h,
                                     start=True, stop=True)
                    nc.vector.tensor_mul(out=dhpre_h[:, j, :], in0=da_ps,
                                         in1=dgel[:, j, :])

                # token-major transposes (fp16) -> staging -> chunk-major DRAM
                st_kc = pA.tile([128, 4, 128], F16, tag="st_kc", bufs=1)
                st_dh = pA.tile([128, 4, 128], F16, tag="st_dh", bufs=1)
                st_dp = pA.tile([128, 4, 512], F16, tag="st_dp", bufs=1)
                st_ha = pA.tile([128, 4, 512], F16, tag="st_ha", bufs=1)
                for blk in range(4):
                    bsl = ts(blk, 128)
                    tp_ps = psA.tile([128, 4, 128], F16, tag="tp", bufs=2)
                    nc.tensor.transpose(tp_ps[:, 0, :], kT_h[:, bsl], ident_h)
                    nc.tensor.transpose(tp_ps[:, 1, :], dhh_h[:, bsl], ident_h)
                    nc.vector.tensor_copy(out=st_kc[:, blk, :], in_=tp_ps[:, 0, :])
                    nc.vector.tensor_copy(out=st_dh[:, blk, :], in_=tp_ps[:, 1, :])
                    for j in range(4):
                        t2_ps = psA.tile([128, 4, 128], F16, tag="tp", bufs=2)
                        nc.tensor.transpose(t2_ps[:, 0, :], dhpre_h[:, j, bsl],
                                            ident_h)
                        nc.tensor.transpose(t2_ps[:, 1, :], hact_h[:, j, bsl],
                                            ident_h)
                        nc.vector.tensor_copy(out=st_dp[:, blk, ts(j, 128)],
                                              in_=t2_ps[:, 0, :])
                        nc.vector.tensor_copy(out=st_ha[:, blk, ts(j, 128)],
                                              in_=t2_ps[:, 1, :])
                for cm, stg in [(kc_st, st_kc), (dhh_st, st_dh),
                                (dhpre_st, st_dp), (hact_st, st_ha)]:
                    v = cm.rearrange("p (a two) x -> p a two x", two=2)
                    dma(v[:, 4 * tt:4 * tt + 4, 0, :], stg[0:64, :, :])
                    dma(v[:, 4 * tt:4 * tt + 4, 1, :], stg[64:128, :, :])

            # finish mom/dec (all chunks)
            mds = pA.tile([2, NCH], F32, tag="mds")
            nc.scalar.activation(out=mds, in_=mdraw, func=AF.Sigmoid,
                                 scale=1.0 / CHUNK, bias=bias_md)
            mrow_f = pA.tile([1, NCH], F32, tag="mrow_f")
            drow_f = pA.tile([1, NCH], F32, tag="drow_f")
            dma(mrow_f, mds[0:1, :])
            dma(drow_f, mds[1:2, :])
            mrow = pA.tile([1, NCH], F16, tag="mrow")
            drow = pA.tile([1, NCH], F16, tag="drow")
            nc.scalar.copy(out=mrow, in_=mrow_f)
            nc.scalar.copy(out=drow, in_=drow_f)
            mb_ps = psA.tile([128, 512], F32, tag="bc", bufs=2)
            nc.tensor.matmul(mb_ps[:, 0:NCH], ones_row_h, mrow, start=True, stop=True)
            nc.tensor.matmul(mb_ps[:, 64:64 + NCH], ones_row_h, drow,
                             start=True, stop=True)
            nc.vector.tensor_copy(out=momB, in_=mb_ps[:, 0:NCH])
            nc.scalar.activation(out=decm1B, in_=mb_ps[:, 64:64 + NCH],
                                 func=AF.Identity, scale=-1.0, bias=1.0)
            nc.vector.tensor_scalar_mul(out=gG, in0=gG, scalar1=-2.0 / DH)

        # ================= PHASE B: grads + sigma-domain NS5 =====================
        with tc.tile_pool(name="phB", bufs=1) as pB, \
             tc.tile_pool(name="psB", bufs=1, space="PSUM") as psB:
            for g in range(NCH // NGRP):
                chs = list(range(g * NGRP, (g + 1) * NGRP))
                n_inst = 2 * NGRP
                gsl = ts(g, NGRP)
                kc_g = pB.tile([64, NGRP, 128], F16, tag="kc_g", bufs=2)
                dma(kc_g, kc_st[:, gsl, :])
                dhh_g = pB.tile([64, NGRP, 128], F16, tag="dhh_g", bufs=2)
                dma(dhh_g, dhh_st[:, gsl, :])
                dhpre_g = pB.tile([64, NGRP, 512], F16, tag="dhpre_g", bufs=2)
                dma(dhpre_g, dhpre_st[:, gsl, :])
                hact_g = pB.tile([64, NGRP, 512], F16, tag="hact_g", bufs=2)
                dma(hact_g, hact_st[:, gsl, :])
                R = pB.tile([128, n_inst], F32, tag="R", bufs=2)
                gsb = pB.tile([128, n_inst, 512], F16, tag="gsb", bufs=1)
                for ii, c in enumerate(chs):
                    kc_l = kc_g[:, ii, :]
                    dhp_l = dhpre_g[:, ii, :]
                    dhh_l = dhh_g[:, ii, :]
                    ha_l = hact_g[:, ii, :]
                    g_ps = psB.tile([128, 512], F32, tag="g", bufs=2)
                    nc.tensor.matmul(g_ps, kc_l, dhp_l, start=True, stop=True)
                    nc.vector.tensor_copy(out=gsb[:, 2 * ii, :], in_=g_ps)
                    scr = pB.tile([128, 512], F16, tag="scr", bufs=2)
                    nc.vector.scalar_tensor_tensor(
                        out=scr, in0=gsb[:, 2 * ii, :], scalar=1.0,
                        in1=gsb[:, 2 * ii, :], op0=AX.mult, op1=AX.mult,
                        accum_out=R[:, 2 * ii:2 * ii + 1])
                    g2_ps = psB.tile([128, 512], F32, tag="g", bufs=2)
                    nc.tensor.matmul(g2_ps, dhh_l, ha_l, start=True, stop=True)
                    nc.vector.tensor_copy(out=gsb[:, 2 * ii + 1, :], in_=g2_ps)
                    scr2 = pB.tile([128, 512], F16, tag="scr", bufs=2)
                    nc.vector.scalar_tensor_tensor(
                        out=scr2, in0=gsb[:, 2 * ii + 1, :], scalar=1.0,
                        in1=gsb[:, 2 * ii + 1, :], op0=AX.mult, op1=AX.mult,
                        accum_out=R[:, 2 * ii + 1:2 * ii + 2])
                # norms
                Rh = pB.tile([128, n_inst], F16, tag="Rh", bufs=2)
                nc.vector.tensor_copy(out=Rh, in_=R)
                nrm_ps = psB.tile([1, n_inst], F32, tag="nrm", bufs=2)
                for i2 in range(n_inst):
                    nc.tensor.matmul(nrm_ps[:, i2:i2 + 1], ones_col_h,
                                     Rh[:, i2:i2 + 1], start=True, stop=True)
                inv2 = pB.tile([1, n_inst], F32, tag="inv2", bufs=2)
                nc.vector.reciprocal(out=inv2, in_=nrm_ps)
                ninv = pB.tile([1, n_inst], F32, tag="ninv", bufs=2)
                nc.scalar.activation(out=ninv, in_=inv2, func=AF.Sqrt)
                nc.scalar.activation(out=ninv, in_=ninv, func=AF.Copy, scale=-1.0)
                nb = pB.tile([128, n_inst], F32, tag="nb", bufs=2)
                nc.gpsimd.partition_broadcast(nb, ninv)

                # direct sigma-domain NS5 on t = -g/nrm (fp16, stable)
                for i2 in range(n_inst):
                    c = chs[i2 // 2]
                    tP = pB.tile([128, 512], F16, tag="tP", bufs=2)
                    nc.vector.tensor_scalar_mul(out=tP, in0=gsb[:, i2, :],
                                                scalar1=nb[:, i2:i2 + 1])
                    tT = pB.tile([128, 4, 128], F16, tag="tT", bufs=2)
                    for j in range(4):
                        tt_ps = psB.tile([128, 128], F16, tag="ttp", bufs=2)
                        nc.tensor.transpose(tt_ps, tP[:, ts(j, 128)], ident_h)
                        nc.vector.tensor_copy(out=tT[:, j, :], in_=tt_ps)
                    for k in range(5):
                        A_ps = psB.tile([128, 128], F32, tag="x2", bufs=2)
                        for j in range(4):
                            nc.tensor.matmul(A_ps, tT[:, j, :], tT[:, j, :],
                                             start=(j == 0), stop=(j == 3))
                        Ab = pB.tile([128, 128], F16, tag="Ab", bufs=2)
                        nc.vector.tensor_scalar_mul(out=Ab, in0=A_ps, scalar1=NSB)
                        Au = pB.tile([128, 128], F16, tag="Au", bufs=2)
                        nc.vector.tensor_copy(out=Au, in_=A_ps)
                        A2_ps = psB.tile([128, 128], F32, tag="x2", bufs=2)
                        nc.tensor.matmul(A2_ps, Ab, Au, start=True, stop=True)
                        Bm = pB.tile([128, 128], F16, tag="Bm", bufs=2)
                        # Bm = (b*A2)*(c/b) + b*A = c*A2 + b*A
                        nc.vector.scalar_tensor_tensor(
                            out=Bm, in0=A2_ps, scalar=NSC / NSB, in1=Ab,
                            op0=AX.mult, op1=AX.add)
                        Bt_ps = psB.tile([128, 512], F32, tag="g", bufs=2)
                        nc.tensor.matmul(Bt_ps, Bm, tP, start=True, stop=True)
                        tPn = pB.tile([128, 512], F16, tag="tP", bufs=2)
                        nc.vector.scalar_tensor_tensor(
                            out=tPn, in0=tP, scalar=NSA, in1=Bt_ps,
                            op0=AX.mult, op1=AX.add)
                        tP = tPn
                        if k < 4:
                            tT = pB.tile([128, 4, 128], F16, tag="tT", bufs=2)
                            for j in range(4):
                                tt_ps = psB.tile([128, 128], F16, tag="ttp", bufs=2)
                                nc.tensor.transpose(tt_ps, tP[:, ts(j, 128)],
                                                    ident_h)
                                nc.vector.tensor_copy(out=tT[:, j, :], in_=tt_ps)
                    if i2 % 2 == 0:
                        dma(s1_st[c], tP)
                    else:
                        # matrix 2: store native (hid, dh) layout via transpose
                        s2n = pB.tile([128, 4, 128], F16, tag="s2n", bufs=2)
                        for j in range(4):
                            tt_ps = psB.tile([128, 128], F16, tag="ttp", bufs=2)
                            nc.tensor.transpose(tt_ps, tP[:, ts(j, 128)], ident_h)
                            nc.vector.tensor_copy(out=s2n[:, j, :], in_=tt_ps)
                        dma(s2_st[c], s2n.rearrange("p a b -> p (a b)"))

        # ================= PHASE C: scans + retrieval + output ================
        with tc.tile_pool(name="phC", bufs=1) as pC, \
             tc.tile_pool(name="psC", bufs=1, space="PSUM") as psC:
            u1 = pC.tile([128, 512], F32, tag="u1")
            u2 = pC.tile([128, 4, 128], F32, tag="u2")
            m1s = pC.tile([128, 512], F32, tag="m1s")
            m2s = pC.tile([128, 4, 128], F32, tag="m2s")
            u1h = pC.tile([128, 512], F16, tag="u1h")
            u2h = pC.tile([128, 4, 128], F16, tag="u2h")
            ugv = pC.tile([128, 1], F32, tag="ugv")
            mgv = pC.tile([128, 1], F32, tag="mgv")
            outT = pC.tile([128, N], F16, tag="outT")
            nc.vector.tensor_copy(out=u1, in_=w1sb)
            nc.vector.tensor_copy(out=u2, in_=w2_h)
            nc.vector.tensor_copy(out=u1h, in_=w1h16)
            nc.vector.tensor_copy(out=u2h, in_=w2_h)
            nc.vector.tensor_copy(out=ugv, in_=gamma)
            nc.vector.memset(m1s, 0.0)
            nc.vector.memset(m2s, 0.0)
            nc.vector.memset(mgv, 0.0)

            for c in range(NCH):
                sl = ts(c, CHUNK)
                s1c = pC.tile([128, 512], F16, tag="s1c", bufs=4)
                dma(s1c, s1_st[c])
                s2c = pC.tile([128, 4, 128], F16, tag="s2c", bufs=4)
                dma(s2c.rearrange("p a b -> p (a b)"), s2_st[c])

                # retrieval with pre-update state
                hp_ps = psC.tile([128, 4, CHUNK], F32, tag="hp", bufs=2)
                for j in range(4):
                    nc.tensor.matmul(hp_ps[:, j, :], u1h[:, ts(j, 128)],
                                     qT_h[:, sl], start=True, stop=True)
                ha_c = pC.tile([128, 4, CHUNK], F16, tag="ha_c", bufs=2)
                nc.scalar.activation(out=ha_c, in_=hp_ps, func=AF.Gelu)
                hh_ps = psC.tile([128, CHUNK], F32, tag="csm", bufs=3)
                for j in range(4):
                    nc.tensor.matmul(hh_ps, u2h[:, j, :], ha_c[:, j, :],
                                     start=(j == 0), stop=(j == 3))
                sqc = pC.tile([128, CHUNK], F16, tag="sqc", bufs=2)
                nc.scalar.activation(out=sqc, in_=hh_ps, func=AF.Square)
                ms_ps = psC.tile([1, CHUNK], F32, tag="csm", bufs=3)
                nc.tensor.matmul(ms_ps, ones_col_h, sqc, start=True, stop=True)
                rr = pC.tile([1, CHUNK], F32, tag="rr", bufs=2)
                nc.scalar.activation(out=rr, in_=ms_ps, func=AF.Sqrt,
                                     scale=1.0 / DH, bias=epsT[0:1, :])
                rr2 = pC.tile([1, CHUNK], F32, tag="rr2", bufs=2)
                nc.vector.reciprocal(out=rr2, in_=rr)
                rrh = pC.tile([1, CHUNK], F16, tag="rrh", bufs=2)
                nc.scalar.copy(out=rrh, in_=rr2)
                sb_ps = psC.tile([128, CHUNK], F32, tag="csm", bufs=3)
                nc.tensor.matmul(sb_ps, ones_row_h, rrh, start=True, stop=True)
                hhc = pC.tile([128, CHUNK], F32, tag="hhc", bufs=2)
                nc.scalar.copy(out=hhc, in_=hh_ps)
                yc = pC.tile([128, CHUNK], F32, tag="yc", bufs=2)
                nc.vector.tensor_mul(out=yc, in0=hhc, in1=sb_ps)
                prc = pC.tile([128, CHUNK], F32, tag="prc", bufs=2)
                nc.vector.scalar_tensor_tensor(out=prc, in0=yc, scalar=ugv,
                                               in1=qT_h[:, sl],
                                               op0=AX.mult, op1=AX.add)
                nc.vector.tensor_mul(out=outT[:, sl], in0=prc, in1=gateB[:, sl])

                # scans (s already = NS output)
                nc.vector.scalar_tensor_tensor(out=m1s, in0=m1s,
                                               scalar=momB[:, c:c + 1], in1=s1c,
                                               op0=AX.mult, op1=AX.add)
                nc.vector.scalar_tensor_tensor(out=u1, in0=u1,
                                               scalar=decm1B[:, c:c + 1], in1=m1s,
                                               op0=AX.mult, op1=AX.add)
                nc.scalar.copy(out=u1h, in_=u1)
                nc.vector.scalar_tensor_tensor(out=m2s, in0=m2s,
                                               scalar=momB[:, c:c + 1], in1=s2c,
                                               op0=AX.mult, op1=AX.add)
                nc.vector.scalar_tensor_tensor(out=u2, in0=u2,
                                               scalar=decm1B[:, c:c + 1], in1=m2s,
                                               op0=AX.mult, op1=AX.add)
                nc.scalar.copy(out=u2h, in_=u2)
                nc.vector.scalar_tensor_tensor(out=mgv, in0=mgv,
                                               scalar=momB[:, c:c + 1],
                                               in1=gG[:, c:c + 1],
                                               op0=AX.mult, op1=AX.add)
                nc.vector.scalar_tensor_tensor(out=ugv, in0=ugv,
                                               scalar=decm1B[:, c:c + 1], in1=mgv,
                                               op0=AX.mult, op1=AX.add)

            # final projection -> f16 partial, staged to DRAM for ReduceScatter
            for i in range(4):
                for tt in range(NTT):
                    o_ps = psC.tile([128, 512], F32, tag="sps", bufs=3)
                    nc.tensor.matmul(o_ps, wc_h[:, ts(i, 128)], outT[:, ts(tt, 512)],
                                     start=True, stop=True)
                    osb = pC.tile([128, 512], F16, tag="osb", bufs=3)
                    nc.scalar.copy(out=osb, in_=o_ps)
                    dma(ccin[ts(i, 128), ts(tt, 512)], osb)

            # on-device head sum: each core keeps a [128, N] row-quarter
            nc.gpsimd.collective_compute(
                "ReduceScatter", AX.add, replica_groups=GROUPS,
                ins=[ccin.opt()], outs=[ccout.opt()])
            dma(d["outp"].ap(), ccout)


# ------------------- host side -------------------

def _prep_global(inputs):
    """Build the concatenated 8-core packh global [8*PACKH_ELEMS] f16,
    writing each region in place (single cast/copy pass per datum)."""
    f32, f16 = np.float32, np.float16
    sg = np.asarray(inputs["store_g"], f32)[:, None]
    rg = np.asarray(inputs["retrieve_g"], f32)[:, None]

    def tile128(w):  # (512, X) -> rows grouped as (128, 4, X) -> (128, 4*X)
        w = np.asarray(w, f32)
        return np.ascontiguousarray(
            w.reshape(4, 128, -1).transpose(1, 0, 2).reshape(128, -1))

    seq = np.asarray(inputs["seq"], f32)
    g = np.empty((8, PACKH_ELEMS), f16)
    half0, half1, tails = [], [], []
    for h in range(HEADS):
        hs = slice(h * DH, (h + 1) * DH)
        wk = tile128(sg * np.asarray(inputs["Wk"], f32)[:, hs]).astype(f16)
        wv = tile128(sg * np.asarray(inputs["Wv"], f32)[:, hs]).astype(f16)
        wq = tile128(rg * np.asarray(inputs["Wq"], f32)[:, hs]).astype(f16)
        wsm = tile128(np.stack([
            sg[:, 0] * np.asarray(inputs["W_lr"], f32)[:, h],
            sg[:, 0] * np.asarray(inputs["Wm"], f32)[:, h],
            sg[:, 0] * np.asarray(inputs["Wd"], f32)[:, h],
            rg[:, 0] * np.asarray(inputs["Wgate"], f32)[:, h]], axis=1)).astype(f16)
        w2 = tile128(np.asarray(inputs["mw2"], f32)[h]).astype(f16)
        wc = np.ascontiguousarray(np.asarray(inputs["Wc"], f32)[hs, :]).astype(f16)
        w1 = np.asarray(inputs["mw1"], f32)[h].astype(f16)
        h0 = np.empty((128, WPK_COLS), f16)
        h0[:, 0:512] = wk; h0[:, 512:1024] = wv; h0[:, 1024:1536] = wq
        h0[:, 1536:1552] = 0.0
        h1 = np.empty((128, WPK_COLS), f16)
        h1[:, 0:512] = w2; h1[:, 512:1024] = wc; h1[:, 1024:1536] = w1
        h1[:, 1536:1552] = wsm
        half0.append(h0)
        half1.append(h1)
        tail = np.empty(642, f16)
        tail[0:512] = 0.0
        tail[0:512].reshape(128, 4)[:, 0] = np.float16(
            np.asarray(inputs["b_lr"], f32)[h])
        tail[512] = np.float16(np.asarray(inputs["bm"], f32)[h])
        tail[513] = np.float16(np.asarray(inputs["bd"], f32)[h])
        tail[514:642] = np.asarray(inputs["mgamma"], f32)[h].astype(f16)
        tails.append(tail)

    for c in range(8):
        b, h = c // HEADS, c % HEADS
        v = g[c]
        # cast+transpose straight into the pack region
        v[0:SEQ_ELEMS].reshape(512, 512)[:] = seq[b][512 * h:512 * (h + 1), :].T
        v[SEQ_ELEMS:TAIL_OFF] = (half0[h] if b == 0 else half1[h]).ravel()
        v[TAIL_OFF:] = tails[h]
    return g


def _prep_in_maps(inputs):
    g = _prep_global(inputs)
    return [{"packh": g[c].copy()} for c in range(8)]


_CACHE = {}


def _get_module():
    if "nc" not in _CACHE:
        nc = bacc.Bacc("TRN2", target_bir_lowering=False, debug=False,
                       num_devices=8)
        build(nc)
        nc.compile()
        _CACHE["nc"] = nc
    return _CACHE["nc"]


def _get_executor(g_example):
    """Process-cached sharded executable of the bass_exec custom call.

    Semantics match bass_utils.run_bass_kernel_spmd under axon
    (bass2jax.run_bass_via_pjrt), except: the executable is built once
    (the library rebuilds + retraces its jit per call, ~0.9 s), no zero
    output buffers are donated (the kernel fully writes outp, so
    uninitialized result buffers are fine and the zero upload is
    skipped), and the AOT compile goes through fast_dispatch_compile
    (C++ fast-path dispatch) when available.
    """
    if "exec" in _CACHE:
        return _CACHE["exec"]
    import jax
    import jax.core
    from jax.sharding import Mesh, PartitionSpec
    try:
        from jax.experimental.shard_map import shard_map
    except ImportError:  # newer jax
        from jax import shard_map
    from concourse import bass2jax

    nc = _get_module()
    bass2jax.install_neuronx_cc_hook()
    partition_name = (nc.partition_id_tensor.name
                      if nc.partition_id_tensor else None)
    in_names, out_names, out_avals = [], [], []
    for alloc in nc.m.functions[0].allocations:
        if not isinstance(alloc, mybir.MemoryLocationSet):
            continue
        name = alloc.memorylocations[0].name
        if alloc.kind == "ExternalInput":
            if name != partition_name:
                in_names.append(name)
        elif alloc.kind == "ExternalOutput":
            out_names.append(name)
            out_avals.append(jax.core.ShapedArray(
                tuple(alloc.tensor_shape), mybir.dt.np(alloc.dtype)))
    assert in_names == ["packh"], in_names
    bind_names = in_names + ([partition_name] if partition_name else [])

    def _body(*args):
        ops = list(args)
        if partition_name is not None:
            ops.append(bass2jax.partition_id_tensor())
        return tuple(bass2jax._bass_exec_p.bind(
            *ops, out_avals=tuple(out_avals), in_names=tuple(bind_names),
            out_names=tuple(out_names), lowering_input_output_aliases=(),
            sim_require_finite=True, sim_require_nnan=True, nc=nc))

    devices = jax.devices()[:8]
    assert len(devices) == 8, f"need 8 devices, got {len(jax.devices())}"
    mesh = Mesh(np.asarray(devices), ("core",))
    shmapped = shard_map(_body, mesh=mesh,
                         in_specs=(PartitionSpec("core"),) * len(in_names),
                         out_specs=(PartitionSpec("core"),) * len(out_names),
                         check_rep=False)
    try:
        sharded = bass2jax.fast_dispatch_compile(
            lambda: jax.jit(shmapped, keep_unused=True)
            .lower(g_example).compile())
    except Exception:
        sharded = jax.jit(shmapped, keep_unused=True)
    _CACHE["exec"] = (sharded, in_names, out_names)
    return _CACHE["exec"]


def _run_fast(g):
    gflat = g.reshape(-1)
    sharded, in_names, out_names = _get_executor(gflat)
    out_arrs = sharded(gflat)
    # Pre-issue the device->host fetch at dispatch time so the request is
    # already queued terminal-side when the result lands (hides the tunnel
    # round trip under upload+exec).
    for o in out_arrs:
        try:
            o.copy_to_host_async()
        except Exception:
            pass
    return {nm: np.asarray(out_arrs[i]) for i, nm in enumerate(out_names)}


def kernel(**inputs):
    nc = _get_module()
    g = _prep_global(inputs)
    try:
        outg = _run_fast(g)["outp"]                # [8*128, N] f16
    except Exception:
        from concourse.bass_utils import run_bass_kernel_spmd
        in_maps = [{"packh": g[c].copy()} for c in range(8)]
        res = run_bass_kernel_spmd(nc, in_maps, core_ids=list(range(8)))
        outg = np.concatenate(
            [res.results[c]["outp"] for c in range(8)], axis=0)
    out = np.zeros((B, N, DIM), np.float32)
    for b in range(B):
        out[b] = outg[512 * b:512 * (b + 1)].T.astype(np.float32)
    return out


if __name__ == "__main__":
    dd = np.load("/root/problem/ref_inputs.npz")
    inputs = {k: dd[k] for k in dd.files}
    out = kernel(**inputs)
    exp = np.load("/root/problem/ref_expected.npy")
    err = np.abs(out - exp).max() / np.abs(exp).max()
    rel = np.linalg.norm(out - exp) / np.linalg.norm(exp)
    print(f"absmax-rel: {err:.3e}  l2-rel: {rel:.3e}")
